# revision 20
# baseline (speedup 1.0000x reference)
"""Trainium2 Bass kernel for CBSA (cross-block self-attention) module — v2.

Shapes (hardcoded from the problem spec):
  x: [8, 4096, 512], proj_w/to_out_w: [512, 512], step_rep/step_x: [8,1,1],
  to_out_b: [512].  Output: [8, 4096, 512].

Sharding: data-parallel over batch, 1 batch per NeuronCore (8 cores).

v2 design notes (all driven by the TRN2 cost model):
 - fp8e4 DoubleRow matmuls everywhere possible (0.5 cyc/output-row).
 - Both w orientations kept in fp8: wT computed directly (proj @ x^T),
   w-natural via PE transposes of wT.  d-index is stored INTERLEAVED
   (d = 256g + 2k + j for partition k, ktile j) so DoubleRow contraction
   packing is consistent across dots / pooling / rep_delta / phase 5.
 - Pooling done as a DoubleRow matmul against a constant 0/1 pool matrix.
 - Output computed TRANSPOSED (out^T[c, n]) so to_out bias is a
   per-partition activation bias; host transposes back.
 - exp() outputs written directly as fp8 (empirically ~3e-5 final error).
 - Elementwise PSUM->SBUF streams spread across ACT / DVE / GpSimd.
"""

import numpy as np
import ml_dtypes

import concourse.bass as bass
import concourse.tile as tile
from concourse import bacc, mybir
from concourse import bass_utils

F32 = mybir.dt.float32
BF16 = mybir.dt.bfloat16
FP8 = mybir.dt.float8e4

B = 8
N = 4096
C = 512
HEADS = 8
DH = 64
Q = 64            # pooled tokens
SCALE = DH ** -0.5
NT = N // 128     # 32 token tiles
NU = NT // 2      # 16 token tile-pairs (DoubleRow k-tiles)
PAIRS = HEADS // 2  # 4 head pairs
NS = N // 512     # 8 free-dim slices of 512
MSC = 2.0 ** 14   # fp8 scaling for M (xds carries 1/(s1*s2) which is ~1e-4)

_CACHE = {}


def _build():
    nc = bacc.Bacc("TRN2", target_bir_lowering=False, debug=False, num_devices=B)

    xT_d = nc.dram_tensor("xT", [128, 2, 2, N], FP8, kind="ExternalInput").ap()
    pwT_d = nc.dram_tensor("pwT", [128, 2, 2, C], FP8, kind="ExternalInput").ap()
    poolT_d = nc.dram_tensor("poolT", [128, NU, 2, Q], FP8,
                             kind="ExternalInput").ap()
    twp_d = nc.dram_tensor("twp", [128, PAIRS, C], BF16,
                           kind="ExternalInput").ap()
    srep_d = nc.dram_tensor("srep", [128, PAIRS], F32, kind="ExternalInput").ap()
    bcc_d = nc.dram_tensor("biascc", [128, 4], F32, kind="ExternalInput").ap()
    id8_d = nc.dram_tensor("ident8", [128, 128], FP8, kind="ExternalInput").ap()
    idb_d = nc.dram_tensor("identb", [128, 128], BF16, kind="ExternalInput").ap()
    idf_d = nc.dram_tensor("identf", [128, 128], F32, kind="ExternalInput").ap()
    outT_d = nc.dram_tensor("outT", [C, N], BF16, kind="ExternalOutput").ap()

    from contextlib import ExitStack
    with tile.TileContext(nc) as tc:
        with ExitStack() as ctx:
            _body.ctx = ctx
            _body(tc, nc, xT_d, pwT_d, poolT_d, twp_d, srep_d, bcc_d,
                  id8_d, idb_d, idf_d, outT_d)
    nc.compile()
    return nc


def _body(tc, nc, xT_d, pwT_d, poolT_d, twp_d, srep_d, bcc_d,
          id8_d, idb_d, idf_d, outT_d):
    Exp = mybir.ActivationFunctionType.Exp
    Copy = mybir.ActivationFunctionType.Copy
    X = mybir.AxisListType.X
    ADD = mybir.AluOpType.add
    MULT = mybir.AluOpType.mult
    DR = mybir.MatmulPerfMode.DoubleRow

    ctx = _body.ctx
    const = ctx.enter_context(tc.tile_pool(name="const", bufs=1))
    persist = ctx.enter_context(tc.tile_pool(name="persist", bufs=1))
    xs_pool = ctx.enter_context(tc.tile_pool(name="xstream", bufs=3))
    sm = ctx.enter_context(tc.tile_pool(name="small", bufs=2))
    ost = ctx.enter_context(tc.tile_pool(name="ostage", bufs=3))
    ps512 = ctx.enter_context(tc.tile_pool(name="ps512", bufs=3, space="PSUM"))
    pstr = ctx.enter_context(tc.tile_pool(name="pstr", bufs=2, space="PSUM"))
    ps128 = ctx.enter_context(tc.tile_pool(name="ps128", bufs=2, space="PSUM"))
    pstb = ctx.enter_context(tc.tile_pool(name="pstb", bufs=1, space="PSUM"))

    # ---- constants ----
    pwT = const.tile([128, 2, 2, C], FP8, tag="pwT")
    nc.sync.dma_start(pwT[:], pwT_d[:])
    ident8 = const.tile([128, 128], FP8, tag="ident8")
    nc.sync.dma_start(ident8[:], id8_d[:])

    # ---- persistent intermediates ----
    # wT fp8, interleaved d: wTf8[k, g, j, n] holds d = 256g + 2k + j
    wTf8 = persist.tile([128, 2, 2, N], FP8, tag="wTf8")
    # w natural fp8: wN[k_n, u, j_n, d]; n = 128*(2u+j_n)+k_n, d natural
    wN = persist.tile([128, NU, 2, C], FP8, tag="wN", name="wN")
    # exp(dots) natural: edf8[g][k_q, j_p, n]  (pair p = 2g + j_p)
    edf8 = [persist.tile([128, 2, N], FP8, tag=f"ed{g}", name=f"ed{g}")
            for g in range(2)]
    # exp(dots)^T: at[p][k_n, u, j_n, qq]
    at = [persist.tile([128, NU, 2, 128], FP8, tag=f"at{p}", name=f"at{p}")
          for p in range(PAIRS)]
    # M (to_out-folded pooled outputs) fp8, scaled by MSC: Mf8[g][k_q, j_p, c]
    Mf8 = [persist.tile([128, 2, C], FP8, tag=f"Mf8{g}", name=f"Mf8{g}")
           for g in range(2)]

    # ================= Phase 1: wT = proj_w @ x^T  and  wN = x @ proj_w^T
    # Both are fp8 DoubleRow streams off the same xts staging tiles; pwT
    # serves as lhsT for wT (d columns) and as rhs for wN (c contraction).
    ectr = [0]
    for sl2 in range(NS // 2):
        xts = xs_pool.tile([128, 2, 2, 1024], FP8, tag="xs", name="xts")
        nc.sync.dma_start(xts[:], xT_d[:, :, :, sl2 * 1024:(sl2 + 1) * 1024])
        for di in range(4):
            pst = [ps512.tile([128, 512], F32, tag="ps512", name=f"pst{j}")
                   for j in range(2)]
            for g in range(2):
                for s2 in range(2):
                    nc.tensor.matmul(
                        pst[s2][:],
                        pwT[:, g, :, di * 128:(di + 1) * 128],
                        xts[:, g, :, s2 * 512:(s2 + 1) * 512],
                        start=(g == 0), stop=(g == 1),
                        perf_mode=DR)
            for s2 in range(2):
                sl = sl2 * 2 + s2
                nc.scalar.activation(
                    wTf8[:, di // 2, di % 2, sl * 512:(sl + 1) * 512],
                    pst[s2][:], Copy, scale=1.0 / 16.0)
        for tt in range(8):
            t = sl2 * 8 + tt
            wps = ps512.tile([128, 512], F32, tag="ps512", name="wps")
            for g in range(2):
                nc.tensor.matmul(
                    wps[:],
                    xts[:, g, :, tt * 128:(tt + 1) * 128],
                    pwT[:, g, :, :],
                    start=(g == 0), stop=(g == 1),
                    perf_mode=DR)
            dst = wN[:, t // 2, t % 2, :]
            if ectr[0] % 2 == 0:
                nc.vector.tensor_scalar_mul(dst, wps[:], 1.0 / 16.0)
            else:
                nc.scalar.activation(dst, wps[:], Copy, scale=1.0 / 16.0)
            ectr[0] += 1

    # deferred constants
    poolT = const.tile([128, NU, 2, Q], FP8, tag="poolT")
    nc.sync.dma_start(poolT[:], poolT_d[:])
    identb = const.tile([128, 128], BF16, tag="identb")
    nc.sync.dma_start(identb[:], idb_d[:])
    identf = const.tile([128, 128], F32, tag="identf")
    nc.sync.dma_start(identf[:], idf_d[:])
    srep = const.tile([128, PAIRS], F32, tag="srep")
    nc.sync.dma_start(srep[:], srep_d[:])
    twp = const.tile([128, PAIRS, C], BF16, tag="twp")
    nc.sync.dma_start(twp[:], twp_d[:])
    biascc = const.tile([128, 4], F32, tag="biascc")
    nc.sync.dma_start(biascc[:], bcc_d[:])

    # ================= Phase 2: pooled rep via DoubleRow matmul ===========
    # rep[q, (g, j_d, k_d)] = sum_n poolT[n, q] * wN[n, (g, j_d, k_d)] / 64
    psp_t = ps512.tile([128, 512], F32, tag="ps512", name="psp")
    psp = psp_t[0:64, :].rearrange("p (g j k) -> p g j k", g=2, j=2)
    for g in range(2):
        for u in range(NU):
            nc.tensor.matmul(
                psp[:, g, :, :],
                poolT[:, u, :, :],
                wN[:, u, :, g * 256:(g + 1) * 256],
                start=(u == 0), stop=(u == NU - 1),
                perf_mode=DR)
    rep_sb = sm.tile([64, 2, 2, 128], F32, tag="rep_sb")
    nc.vector.tensor_copy(rep_sb[:], psp[:])
    # repT_pack[k_d, g, j_d, q] via f32 PE transposes
    prp_t = ps512.tile([128, 512], F32, tag="ps512", name="prp")
    prp = prp_t[:, 0:256].rearrange("p (a q) -> p a q", a=4)
    for g in range(2):
        for j_d in range(2):
            nc.tensor.transpose(prp[:, g * 2 + j_d, :],
                                rep_sb[:, g, j_d, :], identf[0:64, 0:64])
    repT_pack = sm.tile([128, 2, 2, Q], F32, tag="repTp")
    nc.vector.tensor_copy(
        repT_pack.rearrange("p a b q -> p (a b) q")[:], prp[:])

    # block-diag lhsT for dots, fp8, DoubleRow layout [k, j_d, qq].
    # d = 256g + 128*j_d + k, so pair p (= 2g + pl) lives wholly in j_d = pl:
    # head 2p at k 0:64 -> qq 0:64, head 2p+1 at k 64:128 -> qq 64:128.
    dblk = []
    for p in range(PAIRS):
        g, pl = p // 2, p % 2
        bk = sm.tile([128, 2, 128], FP8, tag=f"dblk{p}")
        nc.vector.memset(bk[:], 0.0)
        nc.vector.tensor_copy(bk[0:64, pl, 0:64],
                              repT_pack[0:64, g, pl, :])
        nc.vector.tensor_copy(bk[64:128, pl, 64:128],
                              repT_pack[64:128, g, pl, :])
        dblk.append(bk)

    # ================= Phase 3: dots + exp (all pairs) ====================
    rc1, ssc = [], []
    for p in range(PAIRS):
        g, pl = p // 2, p % 2
        s1p = sm.tile([128, NS], F32, tag=f"s1parts{p}")
        for s in range(NS):
            dps = ps512.tile([128, 512], F32, tag="ps512", name="dps")
            nc.tensor.matmul(dps[:], dblk[p][:],
                             wTf8[:, g, :, s * 512:(s + 1) * 512],
                             start=True, stop=True, perf_mode=DR)
            nc.scalar.activation(edf8[g][:, pl, s * 512:(s + 1) * 512],
                                 dps[:], Exp, scale=SCALE,
                                 accum_out=s1p[:, s:s + 1])
        s1 = sm.tile([128, 1], F32, tag=f"s1_{p}")
        nc.vector.tensor_reduce(s1[:], s1p[:], X, ADD)
        rc = sm.tile([128, 1], F32, tag=f"rc1_{p}")
        nc.vector.reciprocal(rc[:], s1[:])
        sscp = sm.tile([128, 1], F32, tag=f"ssc_{p}")
        nc.vector.tensor_mul(sscp[:], rc[:], srep[:, p:p + 1])
        rc1.append(rc)
        ssc.append(sscp)

    # ================= Phase 4: attn^T via fp8 PE transposes ==============
    for p in range(PAIRS):
        g, pl = p // 2, p % 2
        for b4 in range(NT // 4):
            t0 = b4 * 4
            tp = pstr.tile([128, 4, 128, 2], FP8, tag="pstr8", name="atp")
            for j in range(4):
                t = t0 + j
                nc.tensor.transpose(tp[:, j, :, 0],
                                    edf8[g][:, pl, t * 128:(t + 1) * 128],
                                    ident8[:])
            u0 = t0 // 2
            nc.vector.tensor_copy(
                at[p][:, u0:u0 + 2, :, :],
                tp[:, :, :, 0].rearrange("p (u j) f -> p u j f", u=2, j=2)[:])

    # ================= Phase 5: per-pair rep_delta + self-attn + M ========
    for p in range(PAIRS):
        g, pl = p // 2, p % 2
        # rep_delta[qq, k] accumulated over token tile-pairs; pair block is
        # ktile j_d = pl of group g, natural d order within the block.
        rd_ps = ps128.tile([128, 128], F32, tag="ps128", name="rd")
        for u in range(NU):
            nc.tensor.matmul(rd_ps[:], at[p][:, u, :, :],
                             wN[:, u, :, p * 128:(p + 1) * 128],
                             start=(u == 0), stop=(u == NU - 1),
                             perf_mode=DR)
        # rep natural (both head row-blocks) via identity matmuls
        rp_ps = ps128.tile([128, 128], F32, tag="ps128", name="rp")
        nc.tensor.matmul(rp_ps[0:64, 0:64],
                         repT_pack[0:64, g, pl, :], identf[0:64, 0:64],
                         start=True, stop=True)
        nc.tensor.matmul(rp_ps[64:128, 64:128],
                         repT_pack[64:128, g, pl, :], identf[64:128, 64:128],
                         start=True, stop=True)
        # reph_new (natural, block-diag) bf16
        rep_pair = sm.tile([128, 128], F32, tag="rep_pair")
        nc.vector.tensor_copy(rep_pair[0:64, 0:64], rp_ps[0:64, 0:64])
        nc.vector.tensor_copy(rep_pair[64:128, 64:128], rp_ps[64:128, 64:128])
        rnat = sm.tile([128, 128], BF16, tag="rnat")
        nc.vector.memset(rnat[:], 0.0)
        for h in range(2):
            r0, r1 = 64 * h, 64 * (h + 1)
            nc.vector.scalar_tensor_tensor(rnat[r0:r1, r0:r1],
                                           rd_ps[r0:r1, r0:r1],
                                           ssc[p][r0:r1, 0:1],
                                           rep_pair[r0:r1, r0:r1], MULT, ADD)
        # reph_new^T
        rtp = pstb.tile([128, 128], BF16, tag="pstrb", name="rtp")
        nc.tensor.transpose(rtp[:], rnat[:], identb[:])
        rnT = sm.tile([128, 128], BF16, tag="rnT")
        nc.vector.tensor_copy(rnT[:], rtp[:])
        # dots2 (block-diag) + exp + row sums
        d2_ps = ps128.tile([128, 128], F32, tag="ps128", name="d2")
        nc.tensor.matmul(d2_ps[:], rnT[:], rnT[:], start=True, stop=True)
        ed2 = sm.tile([128, 128], BF16, tag="ed2")
        nc.vector.memset(ed2[:], 0.0)
        s2 = sm.tile([128, 1], F32, tag="s2")
        for h in range(2):
            r0, r1 = 64 * h, 64 * (h + 1)
            nc.scalar.activation(ed2[r0:r1, r0:r1], d2_ps[r0:r1, r0:r1], Exp,
                                 scale=SCALE, accum_out=s2[r0:r1, 0:1])
        # xds = attn2 @ reph_new, scaled by MSC/(s1*s2)
        xds_ps = ps128.tile([128, 128], F32, tag="ps128", name="xds")
        nc.tensor.matmul(xds_ps[:], ed2[:], rnat[:], start=True, stop=True)
        rc2 = sm.tile([128, 1], F32, tag="rc2")
        nc.vector.reciprocal(rc2[:], s2[:])
        sc = sm.tile([128, 1], F32, tag="sc")
        nc.vector.tensor_mul(sc[:], rc1[p][:], rc2[:])
        xds = sm.tile([128, 128], BF16, tag="xds")
        nc.vector.tensor_scalar(xds[:], xds_ps[:], sc[:], MSC, MULT, MULT)
        # M_pair = xds^T @ twp_pair
        xtp = pstb.tile([128, 128], BF16, tag="pstrb", name="xtp")
        nc.tensor.transpose(xtp[:], xds[:], identb[:])
        xdsT = sm.tile([128, 128], BF16, tag="xdsT")
        nc.vector.tensor_copy(xdsT[:], xtp[:])
        mps = ps512.tile([128, 512], F32, tag="ps512", name="mps")
        nc.tensor.matmul(mps[:], xdsT[:], twp[:, p, :], start=True, stop=True)
        nc.vector.tensor_copy(Mf8[g][:, pl, :], mps[:])

    # ================= Phase 6: out^T = sum_g M_g^T @ ed_g + bias =========
    stage_engines = [nc.scalar, nc.vector]
    for ci in range(4):
        for s2 in range(NS // 2):
            ot = ost.tile([128, 2, 512], BF16, tag="ostage", name="ot")
            for k in range(2):
                s = s2 * 2 + k
                ops = ps512.tile([128, 512], F32, tag="ps512", name="ops")
                for g in range(2):
                    nc.tensor.matmul(ops[:], Mf8[g][:, :, ci * 128:(ci + 1) * 128],
                                     edf8[g][:, :, s * 512:(s + 1) * 512],
                                     start=(g == 0), stop=(g == 1),
                                     perf_mode=DR)
                if (ci * 4 + s2) % 2 == 0:
                    nc.scalar.activation(ot[:, k, :], ops[:],
                                         mybir.ActivationFunctionType.Identity,
                                         scale=1.0 / MSC,
                                         bias=biascc[:, ci:ci + 1])
                else:
                    nc.vector.tensor_scalar(ot[:, k, :], ops[:], 1.0 / MSC,
                                            biascc[:, ci:ci + 1], MULT, ADD)
            nc.sync.dma_start(
                outT_d.rearrange("(a p) n -> p a n", p=128)
                [:, ci, s2 * 1024:(s2 + 1) * 1024],
                ot.rearrange("p a b -> p (a b)")[:])


def _prep_inputs(x, proj_w, step_rep, step_x, to_out_w, to_out_b):
    x = np.asarray(x, dtype=np.float32)
    proj_w = np.asarray(proj_w, dtype=np.float32)
    step_rep = np.asarray(step_rep, dtype=np.float32).reshape(HEADS)
    step_x = np.asarray(step_x, dtype=np.float32).reshape(HEADS)
    to_out_w = np.asarray(to_out_w, dtype=np.float32)
    to_out_b = np.asarray(to_out_b, dtype=np.float32)

    # pwT: [k, g, j, d-col] fp8, c = 256g + 2k + j, free cols = natural d
    # (the [g_out, j_d] chunking of wT is d = 256*g_out + 128*j_d + k,
    #  i.e. plain 128-chunks, so no column permutation is needed)
    pw16 = (proj_w.T * 16.0).reshape(2, 128, 2, C)
    pwT = np.ascontiguousarray(pw16.transpose(1, 0, 2, 3)).astype(
        ml_dtypes.float8_e4m3)

    # poolT: [k_n, u, j_n, q], 1/64 where token n is in pooled cell q
    n_idx = (128 * (2 * np.arange(NU)[:, None] + np.arange(2)[None, :]))[None]
    n_idx = n_idx + np.arange(128)[:, None, None]        # [128, NU, 2]
    q_idx = (n_idx // 512) * 8 + (n_idx % 64) // 8       # cell index
    poolT = np.zeros((128, NU, 2, Q), dtype=np.float32)
    np.put_along_axis(poolT, q_idx[..., None], 1.0 / 64.0, axis=3)
    poolT = poolT.astype(ml_dtypes.float8_e4m3)

    # twp: [r, p, c] with pair p's block = natural d rows p*128 .. p*128+128
    twTs = to_out_w.T * np.repeat(step_x, DH)[:, None]   # [d_global, c_out]
    twp = np.ascontiguousarray(
        twTs.reshape(PAIRS, 128, C).transpose(1, 0, 2)).astype(
        ml_dtypes.bfloat16)

    srep = np.empty((128, PAIRS), dtype=np.float32)
    for p in range(PAIRS):
        srep[0:64, p] = step_rep[2 * p]
        srep[64:128, p] = step_rep[2 * p + 1]

    biascc = np.ascontiguousarray(to_out_b.reshape(4, 128).T.astype(np.float32))

    ident8 = np.eye(128, dtype=ml_dtypes.float8_e4m3)
    identb = np.eye(128, dtype=ml_dtypes.bfloat16)
    identf = np.eye(128, dtype=np.float32)

    shared = {
        "pwT": pwT, "poolT": poolT, "twp": twp, "srep": srep,
        "biascc": biascc, "ident8": ident8, "identb": identb, "identf": identf,
    }
    in_maps = []
    for b in range(B):
        xT = np.ascontiguousarray(
            x[b].T.reshape(2, 128, 2, N).transpose(1, 0, 2, 3)).astype(
            ml_dtypes.float8_e4m3)
        in_maps.append({"xT": xT, **shared})
    return in_maps


def kernel(x, proj_w, step_rep, step_x, to_out_w, to_out_b):
    if "nc" not in _CACHE:
        _CACHE["nc"] = _build()
    nc = _CACHE["nc"]
    in_maps = _prep_inputs(x, proj_w, step_rep, step_x, to_out_w, to_out_b)
    res = bass_utils.run_bass_kernel_spmd(nc, in_maps, core_ids=list(range(B)))
    return np.stack(
        [np.asarray(res.results[b]["outT"]).astype(np.float32).T
         for b in range(B)], axis=0)


# revision 21
# speedup vs baseline: 1.0065x; 1.0065x over previous
"""Trainium2 Bass kernel for CBSA (cross-block self-attention) module — v2.

Shapes (hardcoded from the problem spec):
  x: [8, 4096, 512], proj_w/to_out_w: [512, 512], step_rep/step_x: [8,1,1],
  to_out_b: [512].  Output: [8, 4096, 512].

Sharding: data-parallel over batch, 1 batch per NeuronCore (8 cores).

v2 design notes (all driven by the TRN2 cost model):
 - fp8e4 DoubleRow matmuls everywhere possible (0.5 cyc/output-row).
 - Both w orientations kept in fp8: wT computed directly (proj @ x^T),
   w-natural via PE transposes of wT.  d-index is stored INTERLEAVED
   (d = 256g + 2k + j for partition k, ktile j) so DoubleRow contraction
   packing is consistent across dots / pooling / rep_delta / phase 5.
 - Pooling done as a DoubleRow matmul against a constant 0/1 pool matrix.
 - Output computed TRANSPOSED (out^T[c, n]) so to_out bias is a
   per-partition activation bias; host transposes back.
 - exp() outputs written directly as fp8 (empirically ~3e-5 final error).
 - Elementwise PSUM->SBUF streams spread across ACT / DVE / GpSimd.
"""

import numpy as np
import ml_dtypes

import concourse.bass as bass
import concourse.tile as tile
from concourse import bacc, mybir
from concourse import bass_utils

F32 = mybir.dt.float32
BF16 = mybir.dt.bfloat16
FP8 = mybir.dt.float8e4

B = 8
N = 4096
C = 512
HEADS = 8
DH = 64
Q = 64            # pooled tokens
SCALE = DH ** -0.5
NT = N // 128     # 32 token tiles
NU = NT // 2      # 16 token tile-pairs (DoubleRow k-tiles)
PAIRS = HEADS // 2  # 4 head pairs
NS = N // 512     # 8 free-dim slices of 512
MSC = 2.0 ** 14   # fp8 scaling for M (xds carries 1/(s1*s2) which is ~1e-4)

_CACHE = {}


def _build():
    nc = bacc.Bacc("TRN2", target_bir_lowering=False, debug=False, num_devices=B)

    xT_d = nc.dram_tensor("xT", [128, 2, 2, N], FP8, kind="ExternalInput").ap()
    pwT_d = nc.dram_tensor("pwT", [128, 2, 2, C], FP8, kind="ExternalInput").ap()
    poolT_d = nc.dram_tensor("poolT", [128, NU, 2, Q], FP8,
                             kind="ExternalInput").ap()
    twp_d = nc.dram_tensor("twp", [128, PAIRS, C], BF16,
                           kind="ExternalInput").ap()
    srep_d = nc.dram_tensor("srep", [128, PAIRS], F32, kind="ExternalInput").ap()
    bcc_d = nc.dram_tensor("biascc", [128, 4], F32, kind="ExternalInput").ap()
    id8_d = nc.dram_tensor("ident8", [128, 128], FP8, kind="ExternalInput").ap()
    idb_d = nc.dram_tensor("identb", [128, 128], BF16, kind="ExternalInput").ap()
    idf_d = nc.dram_tensor("identf", [128, 128], F32, kind="ExternalInput").ap()
    outT_d = nc.dram_tensor("outT", [C, N], BF16, kind="ExternalOutput").ap()

    from contextlib import ExitStack
    with tile.TileContext(nc) as tc:
        with ExitStack() as ctx:
            _body.ctx = ctx
            _body(tc, nc, xT_d, pwT_d, poolT_d, twp_d, srep_d, bcc_d,
                  id8_d, idb_d, idf_d, outT_d)
    nc.compile()
    return nc


def _body(tc, nc, xT_d, pwT_d, poolT_d, twp_d, srep_d, bcc_d,
          id8_d, idb_d, idf_d, outT_d):
    Exp = mybir.ActivationFunctionType.Exp
    Copy = mybir.ActivationFunctionType.Copy
    X = mybir.AxisListType.X
    ADD = mybir.AluOpType.add
    MULT = mybir.AluOpType.mult
    DR = mybir.MatmulPerfMode.DoubleRow

    ctx = _body.ctx
    const = ctx.enter_context(tc.tile_pool(name="const", bufs=1))
    persist = ctx.enter_context(tc.tile_pool(name="persist", bufs=1))
    xs_pool = ctx.enter_context(tc.tile_pool(name="xstream", bufs=3))
    sm = ctx.enter_context(tc.tile_pool(name="small", bufs=2))
    ost = ctx.enter_context(tc.tile_pool(name="ostage", bufs=3))
    ps512 = ctx.enter_context(tc.tile_pool(name="ps512", bufs=3, space="PSUM"))
    pstr = ctx.enter_context(tc.tile_pool(name="pstr", bufs=2, space="PSUM"))
    ps128 = ctx.enter_context(tc.tile_pool(name="ps128", bufs=2, space="PSUM"))
    pstb = ctx.enter_context(tc.tile_pool(name="pstb", bufs=1, space="PSUM"))

    # ---- constants ----
    pwT = const.tile([128, 2, 2, C], FP8, tag="pwT")
    nc.sync.dma_start(pwT[:], pwT_d[:])
    ident8 = const.tile([128, 128], FP8, tag="ident8")
    nc.sync.dma_start(ident8[:], id8_d[:])

    # ---- persistent intermediates ----
    # wT fp8, interleaved d: wTf8[k, g, j, n] holds d = 256g + 2k + j
    wTf8 = persist.tile([128, 2, 2, N], FP8, tag="wTf8")
    # w natural fp8: wN[k_n, u, j_n, d]; n = 128*(2u+j_n)+k_n, d natural
    wN = persist.tile([128, NU, 2, C], FP8, tag="wN", name="wN")
    # exp(dots) natural: edf8[g][k_q, j_p, n]  (pair p = 2g + j_p)
    edf8 = [persist.tile([128, 2, N], FP8, tag=f"ed{g}", name=f"ed{g}")
            for g in range(2)]
    # exp(dots)^T: at[p][k_n, u, j_n, qq]
    at = [persist.tile([128, NU, 2, 128], FP8, tag=f"at{p}", name=f"at{p}")
          for p in range(PAIRS)]
    # M (to_out-folded pooled outputs) fp8, scaled by MSC: Mf8[g][k_q, j_p, c]
    Mf8 = [persist.tile([128, 2, C], FP8, tag=f"Mf8{g}", name=f"Mf8{g}")
           for g in range(2)]

    # ================= Phase 1: wT = proj_w @ x^T  and  wN = x @ proj_w^T
    # Both are fp8 DoubleRow streams off the same xts staging tiles; pwT
    # serves as lhsT for wT (d columns) and as rhs for wN (c contraction).
    ectr = [0]
    for sl2 in range(NS // 2):
        xts = xs_pool.tile([128, 2, 2, 1024], FP8, tag="xs", name="xts")
        nc.sync.dma_start(xts[:], xT_d[:, :, :, sl2 * 1024:(sl2 + 1) * 1024])
        for di in range(4):
            pst = [ps512.tile([128, 512], F32, tag="ps512", name=f"pst{j}")
                   for j in range(2)]
            for g in range(2):
                for s2 in range(2):
                    nc.tensor.matmul(
                        pst[s2][:],
                        pwT[:, g, :, di * 128:(di + 1) * 128],
                        xts[:, g, :, s2 * 512:(s2 + 1) * 512],
                        start=(g == 0), stop=(g == 1),
                        perf_mode=DR)
            for s2 in range(2):
                sl = sl2 * 2 + s2
                dst = wTf8[:, di // 2, di % 2, sl * 512:(sl + 1) * 512]
                if s2 == 0:
                    nc.scalar.activation(dst, pst[s2][:], Copy,
                                         scale=1.0 / 16.0)
                else:
                    nc.vector.tensor_scalar_mul(dst, pst[s2][:], 1.0 / 16.0)
        for tt in range(8):
            t = sl2 * 8 + tt
            wps = ps512.tile([128, 512], F32, tag="ps512", name="wps")
            for g in range(2):
                nc.tensor.matmul(
                    wps[:],
                    xts[:, g, :, tt * 128:(tt + 1) * 128],
                    pwT[:, g, :, :],
                    start=(g == 0), stop=(g == 1),
                    perf_mode=DR)
            dst = wN[:, t // 2, t % 2, :]
            if ectr[0] % 2 == 0:
                nc.vector.tensor_scalar_mul(dst, wps[:], 1.0 / 16.0)
            else:
                nc.scalar.activation(dst, wps[:], Copy, scale=1.0 / 16.0)
            ectr[0] += 1

    # deferred constants
    poolT = const.tile([128, NU, 2, Q], FP8, tag="poolT")
    nc.sync.dma_start(poolT[:], poolT_d[:])
    identb = const.tile([128, 128], BF16, tag="identb")
    nc.sync.dma_start(identb[:], idb_d[:])
    identf = const.tile([128, 128], F32, tag="identf")
    nc.sync.dma_start(identf[:], idf_d[:])
    srep = const.tile([128, PAIRS], F32, tag="srep")
    nc.sync.dma_start(srep[:], srep_d[:])
    twp = const.tile([128, PAIRS, C], BF16, tag="twp")
    nc.sync.dma_start(twp[:], twp_d[:])
    biascc = const.tile([128, 4], F32, tag="biascc")
    nc.sync.dma_start(biascc[:], bcc_d[:])

    # ================= Phase 2: pooled rep via DoubleRow matmul ===========
    # rep[q, (g, j_d, k_d)] = sum_n poolT[n, q] * wN[n, (g, j_d, k_d)] / 64
    psp_t = ps512.tile([128, 512], F32, tag="ps512", name="psp")
    psp = psp_t[0:64, :].rearrange("p (g j k) -> p g j k", g=2, j=2)
    for g in range(2):
        for u in range(NU):
            nc.tensor.matmul(
                psp[:, g, :, :],
                poolT[:, u, :, :],
                wN[:, u, :, g * 256:(g + 1) * 256],
                start=(u == 0), stop=(u == NU - 1),
                perf_mode=DR)
    rep_sb = sm.tile([64, 2, 2, 128], F32, tag="rep_sb")
    nc.vector.tensor_copy(rep_sb[:], psp[:])
    # repT_pack[k_d, g, j_d, q] via f32 PE transposes
    prp_t = ps512.tile([128, 512], F32, tag="ps512", name="prp")
    prp = prp_t[:, 0:256].rearrange("p (a q) -> p a q", a=4)
    for g in range(2):
        for j_d in range(2):
            nc.tensor.transpose(prp[:, g * 2 + j_d, :],
                                rep_sb[:, g, j_d, :], identf[0:64, 0:64])
    repT_pack = sm.tile([128, 2, 2, Q], F32, tag="repTp")
    nc.vector.tensor_copy(
        repT_pack.rearrange("p a b q -> p (a b) q")[:], prp[:])

    # block-diag lhsT for dots, fp8, DoubleRow layout [k, j_d, qq].
    # d = 256g + 128*j_d + k, so pair p (= 2g + pl) lives wholly in j_d = pl:
    # head 2p at k 0:64 -> qq 0:64, head 2p+1 at k 64:128 -> qq 64:128.
    dblk = []
    for p in range(PAIRS):
        g, pl = p // 2, p % 2
        bk = sm.tile([128, 2, 128], FP8, tag=f"dblk{p}")
        nc.vector.memset(bk[:], 0.0)
        nc.vector.tensor_copy(bk[0:64, pl, 0:64],
                              repT_pack[0:64, g, pl, :])
        nc.vector.tensor_copy(bk[64:128, pl, 64:128],
                              repT_pack[64:128, g, pl, :])
        dblk.append(bk)

    # ================= Phase 3: dots + exp (all pairs) ====================
    rc1, ssc = [], []
    for p in range(PAIRS):
        g, pl = p // 2, p % 2
        s1p = sm.tile([128, NS], F32, tag=f"s1parts{p}")
        for s in range(NS):
            dps = ps512.tile([128, 512], F32, tag="ps512", name="dps")
            nc.tensor.matmul(dps[:], dblk[p][:],
                             wTf8[:, g, :, s * 512:(s + 1) * 512],
                             start=True, stop=True, perf_mode=DR)
            nc.scalar.activation(edf8[g][:, pl, s * 512:(s + 1) * 512],
                                 dps[:], Exp, scale=SCALE,
                                 accum_out=s1p[:, s:s + 1])
        s1 = sm.tile([128, 1], F32, tag=f"s1_{p}")
        nc.vector.tensor_reduce(s1[:], s1p[:], X, ADD)
        rc = sm.tile([128, 1], F32, tag=f"rc1_{p}")
        nc.vector.reciprocal(rc[:], s1[:])
        sscp = sm.tile([128, 1], F32, tag=f"ssc_{p}")
        nc.vector.tensor_mul(sscp[:], rc[:], srep[:, p:p + 1])
        rc1.append(rc)
        ssc.append(sscp)

    # ================= Phase 4: attn^T via fp8 PE transposes ==============
    for p in range(PAIRS):
        g, pl = p // 2, p % 2
        for b4 in range(NT // 4):
            t0 = b4 * 4
            tp = pstr.tile([128, 4, 128, 2], FP8, tag="pstr8", name="atp")
            for j in range(4):
                t = t0 + j
                nc.tensor.transpose(tp[:, j, :, 0],
                                    edf8[g][:, pl, t * 128:(t + 1) * 128],
                                    ident8[:])
            u0 = t0 // 2
            nc.vector.tensor_copy(
                at[p][:, u0:u0 + 2, :, :],
                tp[:, :, :, 0].rearrange("p (u j) f -> p u j f", u=2, j=2)[:])

    # ================= Phase 5: per-pair rep_delta + self-attn + M ========
    for p in range(PAIRS):
        g, pl = p // 2, p % 2
        # rep_delta[qq, k] accumulated over token tile-pairs; pair block is
        # ktile j_d = pl of group g, natural d order within the block.
        rd_ps = ps128.tile([128, 128], F32, tag="ps128", name="rd")
        for u in range(NU):
            nc.tensor.matmul(rd_ps[:], at[p][:, u, :, :],
                             wN[:, u, :, p * 128:(p + 1) * 128],
                             start=(u == 0), stop=(u == NU - 1),
                             perf_mode=DR)
        # rep natural (both head row-blocks) via identity matmuls
        rp_ps = ps128.tile([128, 128], F32, tag="ps128", name="rp")
        nc.tensor.matmul(rp_ps[0:64, 0:64],
                         repT_pack[0:64, g, pl, :], identf[0:64, 0:64],
                         start=True, stop=True)
        nc.tensor.matmul(rp_ps[64:128, 64:128],
                         repT_pack[64:128, g, pl, :], identf[64:128, 64:128],
                         start=True, stop=True)
        # reph_new (natural, block-diag) bf16
        rep_pair = sm.tile([128, 128], F32, tag="rep_pair")
        nc.vector.tensor_copy(rep_pair[0:64, 0:64], rp_ps[0:64, 0:64])
        nc.vector.tensor_copy(rep_pair[64:128, 64:128], rp_ps[64:128, 64:128])
        rnat = sm.tile([128, 128], BF16, tag="rnat")
        nc.vector.memset(rnat[:], 0.0)
        for h in range(2):
            r0, r1 = 64 * h, 64 * (h + 1)
            nc.vector.scalar_tensor_tensor(rnat[r0:r1, r0:r1],
                                           rd_ps[r0:r1, r0:r1],
                                           ssc[p][r0:r1, 0:1],
                                           rep_pair[r0:r1, r0:r1], MULT, ADD)
        # reph_new^T
        rtp = pstb.tile([128, 128], BF16, tag="pstrb", name="rtp")
        nc.tensor.transpose(rtp[:], rnat[:], identb[:])
        rnT = sm.tile([128, 128], BF16, tag="rnT")
        nc.vector.tensor_copy(rnT[:], rtp[:])
        # dots2 (block-diag) + exp + row sums
        d2_ps = ps128.tile([128, 128], F32, tag="ps128", name="d2")
        nc.tensor.matmul(d2_ps[:], rnT[:], rnT[:], start=True, stop=True)
        ed2 = sm.tile([128, 128], BF16, tag="ed2")
        nc.vector.memset(ed2[:], 0.0)
        s2 = sm.tile([128, 1], F32, tag="s2")
        for h in range(2):
            r0, r1 = 64 * h, 64 * (h + 1)
            nc.scalar.activation(ed2[r0:r1, r0:r1], d2_ps[r0:r1, r0:r1], Exp,
                                 scale=SCALE, accum_out=s2[r0:r1, 0:1])
        # xds = attn2 @ reph_new, scaled by MSC/(s1*s2)
        xds_ps = ps128.tile([128, 128], F32, tag="ps128", name="xds")
        nc.tensor.matmul(xds_ps[:], ed2[:], rnat[:], start=True, stop=True)
        rc2 = sm.tile([128, 1], F32, tag="rc2")
        nc.vector.reciprocal(rc2[:], s2[:])
        sc = sm.tile([128, 1], F32, tag="sc")
        nc.vector.tensor_mul(sc[:], rc1[p][:], rc2[:])
        xds = sm.tile([128, 128], BF16, tag="xds")
        nc.vector.tensor_scalar(xds[:], xds_ps[:], sc[:], MSC, MULT, MULT)
        # M_pair = xds^T @ twp_pair
        xtp = pstb.tile([128, 128], BF16, tag="pstrb", name="xtp")
        nc.tensor.transpose(xtp[:], xds[:], identb[:])
        xdsT = sm.tile([128, 128], BF16, tag="xdsT")
        nc.vector.tensor_copy(xdsT[:], xtp[:])
        mps = ps512.tile([128, 512], F32, tag="ps512", name="mps")
        nc.tensor.matmul(mps[:], xdsT[:], twp[:, p, :], start=True, stop=True)
        nc.vector.tensor_copy(Mf8[g][:, pl, :], mps[:])

    # ================= Phase 6: out^T = sum_g M_g^T @ ed_g + bias =========
    stage_engines = [nc.scalar, nc.vector]
    for ci in range(4):
        for s2 in range(NS // 2):
            ot = ost.tile([128, 2, 512], BF16, tag="ostage", name="ot")
            for k in range(2):
                s = s2 * 2 + k
                ops = ps512.tile([128, 512], F32, tag="ps512", name="ops")
                for g in range(2):
                    nc.tensor.matmul(ops[:], Mf8[g][:, :, ci * 128:(ci + 1) * 128],
                                     edf8[g][:, :, s * 512:(s + 1) * 512],
                                     start=(g == 0), stop=(g == 1),
                                     perf_mode=DR)
                if (ci * 4 + s2) % 2 == 0:
                    nc.scalar.activation(ot[:, k, :], ops[:],
                                         mybir.ActivationFunctionType.Identity,
                                         scale=1.0 / MSC,
                                         bias=biascc[:, ci:ci + 1])
                else:
                    nc.vector.tensor_scalar(ot[:, k, :], ops[:], 1.0 / MSC,
                                            biascc[:, ci:ci + 1], MULT, ADD)
            nc.sync.dma_start(
                outT_d.rearrange("(a p) n -> p a n", p=128)
                [:, ci, s2 * 1024:(s2 + 1) * 1024],
                ot.rearrange("p a b -> p (a b)")[:])


def _prep_inputs(x, proj_w, step_rep, step_x, to_out_w, to_out_b):
    x = np.asarray(x, dtype=np.float32)
    proj_w = np.asarray(proj_w, dtype=np.float32)
    step_rep = np.asarray(step_rep, dtype=np.float32).reshape(HEADS)
    step_x = np.asarray(step_x, dtype=np.float32).reshape(HEADS)
    to_out_w = np.asarray(to_out_w, dtype=np.float32)
    to_out_b = np.asarray(to_out_b, dtype=np.float32)

    # pwT: [k, g, j, d-col] fp8, c = 256g + 2k + j, free cols = natural d
    # (the [g_out, j_d] chunking of wT is d = 256*g_out + 128*j_d + k,
    #  i.e. plain 128-chunks, so no column permutation is needed)
    pw16 = (proj_w.T * 16.0).reshape(2, 128, 2, C)
    pwT = np.ascontiguousarray(pw16.transpose(1, 0, 2, 3)).astype(
        ml_dtypes.float8_e4m3)

    # poolT: [k_n, u, j_n, q], 1/64 where token n is in pooled cell q
    n_idx = (128 * (2 * np.arange(NU)[:, None] + np.arange(2)[None, :]))[None]
    n_idx = n_idx + np.arange(128)[:, None, None]        # [128, NU, 2]
    q_idx = (n_idx // 512) * 8 + (n_idx % 64) // 8       # cell index
    poolT = np.zeros((128, NU, 2, Q), dtype=np.float32)
    np.put_along_axis(poolT, q_idx[..., None], 1.0 / 64.0, axis=3)
    poolT = poolT.astype(ml_dtypes.float8_e4m3)

    # twp: [r, p, c] with pair p's block = natural d rows p*128 .. p*128+128
    twTs = to_out_w.T * np.repeat(step_x, DH)[:, None]   # [d_global, c_out]
    twp = np.ascontiguousarray(
        twTs.reshape(PAIRS, 128, C).transpose(1, 0, 2)).astype(
        ml_dtypes.bfloat16)

    srep = np.empty((128, PAIRS), dtype=np.float32)
    for p in range(PAIRS):
        srep[0:64, p] = step_rep[2 * p]
        srep[64:128, p] = step_rep[2 * p + 1]

    biascc = np.ascontiguousarray(to_out_b.reshape(4, 128).T.astype(np.float32))

    ident8 = np.eye(128, dtype=ml_dtypes.float8_e4m3)
    identb = np.eye(128, dtype=ml_dtypes.bfloat16)
    identf = np.eye(128, dtype=np.float32)

    shared = {
        "pwT": pwT, "poolT": poolT, "twp": twp, "srep": srep,
        "biascc": biascc, "ident8": ident8, "identb": identb, "identf": identf,
    }
    in_maps = []
    for b in range(B):
        xT = np.ascontiguousarray(
            x[b].T.reshape(2, 128, 2, N).transpose(1, 0, 2, 3)).astype(
            ml_dtypes.float8_e4m3)
        in_maps.append({"xT": xT, **shared})
    return in_maps


def kernel(x, proj_w, step_rep, step_x, to_out_w, to_out_b):
    if "nc" not in _CACHE:
        _CACHE["nc"] = _build()
    nc = _CACHE["nc"]
    in_maps = _prep_inputs(x, proj_w, step_rep, step_x, to_out_w, to_out_b)
    res = bass_utils.run_bass_kernel_spmd(nc, in_maps, core_ids=list(range(B)))
    return np.stack(
        [np.asarray(res.results[b]["outT"]).astype(np.float32).T
         for b in range(B)], axis=0)
